# revision 3
# baseline (speedup 1.0000x reference)
"""Trainium2 Bass kernel for the contrastive loss:

    epos = exp(cos_sim(q_pos, img_pos))   # [2B] rows, D=1024
    eneg = exp(cos_sim(q_neg, img_neg))   # [23B]
    pos_sum = segsum(epos, 2); neg_sum = segsum(eneg, 23)   # [B]
    loss = sum((neg_sum - pos_sum) / (pos_sum + neg_sum + 0.001))

Data-parallel over 8 NeuronCores: core c takes batch items [c*512, (c+1)*512),
i.e. rows [c*1024,(c+1)*1024) of the pos tensors and [c*11776,(c+1)*11776) of
the neg tensors. Each core emits its 512 per-item values; the host sums.

Per-core layout: local item i = 4*p + s (partition p in [0,128), slot s in
[0,4)), so partition p owns pos rows 8p..8p+7 and neg rows 92p..92p+91 of the
core's shard — each partition's rows are contiguous in DRAM, so every DMA is
128 partitions x (4 rows * 4KiB) contiguous.

Per 128-row slice [128, 1024]: the row-wise dot runs on the vector engine as
one fused scalar_tensor_tensor ((a*1)*b with accum_out), and the two
sum-of-squares run on the scalar engine as Square activations with accum_out.
A fraction of the b-squares is moved to the vector engine to balance the two
engines; both stay below the DMA floor (~100 MiB/core through 16 SDMA
engines).

cos and e=exp(cos) are computed per chunk as stats complete, using
1/sqrt(x) = exp(-0.5*ln(x)) so the scalar engine needs only the
natural_log_exp_and_others table set (square/ln/exp) for the entire kernel —
no ~2.7us ACT table switches in the final tail. The tail is just the two
segmented reductions and the per-item fixup.
"""

import numpy as np

import concourse.bass as bass
import concourse.tile as tile
from concourse import mybir
from concourse.bass_utils import run_bass_kernel_spmd

EPS_COS = 1e-8
EP = 0.001

N_CORES = 8
P = 128            # SBUF partitions
D = 1024           # embedding dim
B_FULL = 4096      # total batch items
ITEMS = B_FULL // N_CORES   # 512 items per core
SLOTS = ITEMS // P          # 4 items per partition
J_POS = SLOTS * 2           # 8 pos rows per partition
J_NEG = SLOTS * 23          # 92 neg rows per partition
G = 4                       # j-slices per DMA chunk (2 MiB per tensor)

F32 = mybir.dt.float32
ALU = mybir.AluOpType
ACTF = mybir.ActivationFunctionType


def _split_multiwait_instructions(nc):
    """The walrus build here rejects >1 sync-wait per instruction; hoist extra
    waits onto single-wait NOPs placed just before the instruction."""
    ctr = 0
    for fn in nc.m.functions:
        for bb in fn.blocks:
            insts = list(bb.instructions)
            if not any(
                i.sync_info is not None and len(i.sync_info.on_wait) > 1
                for i in insts
            ):
                continue
            new_insts = []
            for inst in insts:
                si = inst.sync_info
                if si is not None and len(si.on_wait) > 1:
                    waits = list(si.on_wait)
                    is_drain = type(inst).__name__ == "InstDrain"
                    keep = [] if is_drain else waits[-1:]
                    move = waits if is_drain else waits[:-1]
                    for w in move:
                        ctr += 1
                        new_insts.append(
                            mybir.InstNoOp(
                                name=f"I-wsplit-{ctr}",
                                engine=inst.engine,
                                sync_info=mybir.SyncInfo(on_wait=[w], on_update=[]),
                                text_hint="wsplit",
                            )
                        )
                    si.on_wait = keep
                new_insts.append(inst)
            bb.instructions = new_insts


def build_bass():
    nc = bass.Bass()
    qp = nc.declare_dram_parameter("qp", [P * J_POS, D], F32, isOutput=False)
    pi = nc.declare_dram_parameter("pi", [P * J_POS, D], F32, isOutput=False)
    qn = nc.declare_dram_parameter("qn", [P * J_NEG, D], F32, isOutput=False)
    ni = nc.declare_dram_parameter("ni", [P * J_NEG, D], F32, isOutput=False)
    out = nc.declare_dram_parameter("out", [P, SLOTS], F32, isOutput=True)

    qp_v = qp[:].rearrange("(p j) d -> p j d", j=J_POS)
    pi_v = pi[:].rearrange("(p j) d -> p j d", j=J_POS)
    qn_v = qn[:].rearrange("(p j) d -> p j d", j=J_NEG)
    ni_v = ni[:].rearrange("(p j) d -> p j d", j=J_NEG)

    with tile.TileContext(nc) as tc:
        with (
            tc.tile_pool(name="io", bufs=4) as io,
            tc.tile_pool(name="st", bufs=1) as st,
            tc.tile_pool(name="cols", bufs=3) as cols,
        ):
            dot_p = st.tile([P, J_POS], F32)
            na2_p = st.tile([P, J_POS], F32)
            nb2_p = st.tile([P, J_POS], F32)
            dot_n = st.tile([P, J_NEG], F32)
            na2_n = st.tile([P, J_NEG], F32)
            nb2_n = st.tile([P, J_NEG], F32)
            e_p = st.tile([P, J_POS], F32)
            e_n = st.tile([P, J_NEG], F32)
            scr_v = st.tile([P, D], F32)
            scr_s = st.tile([P, D], F32)

            # (a_view, b_view, dots, na2, nb2, e, j0) per chunk of G slices
            chunks = []
            for c in range(J_POS // G):
                chunks.append((qp_v, pi_v, dot_p, na2_p, nb2_p, e_p, c * G))
            for c in range(J_NEG // G):
                chunks.append((qn_v, ni_v, dot_n, na2_n, nb2_n, e_n, c * G))

            slice_idx = 0
            for a_v, b_v, dots, na2s, nb2s, e_out, j0 in chunks:
                a_t = io.tile([P, G, D], F32, tag="a")
                b_t = io.tile([P, G, D], F32, tag="b")
                nc.sync.dma_start(out=a_t, in_=a_v[:, j0 : j0 + G, :])
                nc.scalar.dma_start(out=b_t, in_=b_v[:, j0 : j0 + G, :])
                for g in range(G):
                    j = j0 + g
                    a_sl = a_t[:, g, :]
                    b_sl = b_t[:, g, :]
                    nc.vector.scalar_tensor_tensor(
                        out=scr_v[:], in0=a_sl, scalar=1.0, in1=b_sl,
                        op0=ALU.mult, op1=ALU.mult,
                        accum_out=dots[:, j : j + 1],
                    )
                    nc.scalar.activation(
                        out=scr_s[:], in_=a_sl, func=ACTF.Square,
                        accum_out=na2s[:, j : j + 1],
                    )
                    # ~3/8 of b-squares on the vector engine balances DVE/ACT
                    if (slice_idx % 8) < 3:
                        nc.vector.scalar_tensor_tensor(
                            out=scr_v[:], in0=b_sl, scalar=1.0, in1=b_sl,
                            op0=ALU.mult, op1=ALU.mult,
                            accum_out=nb2s[:, j : j + 1],
                        )
                    else:
                        nc.scalar.activation(
                            out=scr_s[:], in_=b_sl, func=ACTF.Square,
                            accum_out=nb2s[:, j : j + 1],
                        )
                    slice_idx += 1

                # e[:, j0:j0+G] = exp(dot * exp(-0.5*ln(max(na2*nb2, eps^2))))
                dsl = slice(j0, j0 + G)
                prod = cols.tile([P, G], F32, tag="prod")
                nc.vector.tensor_tensor(
                    out=prod[:], in0=na2s[:, dsl], in1=nb2s[:, dsl], op=ALU.mult
                )
                nc.vector.tensor_scalar(
                    out=prod[:], in0=prod[:], scalar1=EPS_COS * EPS_COS,
                    scalar2=None, op0=ALU.max,
                )
                nc.scalar.activation(out=prod[:], in_=prod[:], func=ACTF.Ln)
                nc.scalar.activation(
                    out=prod[:], in_=prod[:], func=ACTF.Exp, scale=-0.5
                )
                cosv = cols.tile([P, G], F32, tag="cos")
                nc.vector.tensor_tensor(
                    out=cosv[:], in0=dots[:, dsl], in1=prod[:], op=ALU.mult
                )
                nc.scalar.activation(
                    out=e_out[:, dsl], in_=cosv[:], func=ACTF.Exp
                )

            pos_sum = st.tile([P, SLOTS], F32)
            neg_sum = st.tile([P, SLOTS], F32)
            nc.vector.tensor_reduce(
                out=pos_sum[:],
                in_=e_p[:].rearrange("p (s t) -> p s t", t=2),
                axis=mybir.AxisListType.X,
                op=ALU.add,
            )
            nc.vector.tensor_reduce(
                out=neg_sum[:],
                in_=e_n[:].rearrange("p (s t) -> p s t", t=23),
                axis=mybir.AxisListType.X,
                op=ALU.add,
            )
            num = st.tile([P, SLOTS], F32)
            den = st.tile([P, SLOTS], F32)
            nc.vector.tensor_tensor(
                out=num[:], in0=neg_sum[:], in1=pos_sum[:], op=ALU.subtract
            )
            nc.vector.tensor_tensor(
                out=den[:], in0=neg_sum[:], in1=pos_sum[:], op=ALU.add
            )
            nc.vector.tensor_scalar(
                out=den[:], in0=den[:], scalar1=EP, scalar2=None, op0=ALU.add
            )
            rden = st.tile([P, SLOTS], F32)
            nc.vector.reciprocal(out=rden[:], in_=den[:])
            per_item = st.tile([P, SLOTS], F32)
            nc.vector.tensor_tensor(
                out=per_item[:], in0=num[:], in1=rden[:], op=ALU.mult
            )
            nc.sync.dma_start(out=out[:], in_=per_item[:])

    _split_multiwait_instructions(nc)
    return nc


_NC_CACHE = None


def _get_nc():
    global _NC_CACHE
    if _NC_CACHE is None:
        _NC_CACHE = build_bass()
    return _NC_CACHE


def kernel(question_embeddings_pos, question_embeddings_neg,
           pos_image_embeddings, neg_image_embeddings, batch_size=None,
           **_unused):
    qp = np.ascontiguousarray(np.asarray(question_embeddings_pos, dtype=np.float32))
    qn = np.ascontiguousarray(np.asarray(question_embeddings_neg, dtype=np.float32))
    pi = np.ascontiguousarray(np.asarray(pos_image_embeddings, dtype=np.float32))
    ni = np.ascontiguousarray(np.asarray(neg_image_embeddings, dtype=np.float32))

    rp = 2 * ITEMS   # pos rows per core
    rn = 23 * ITEMS  # neg rows per core
    in_maps = [
        {
            "qp": qp[c * rp : (c + 1) * rp],
            "pi": pi[c * rp : (c + 1) * rp],
            "qn": qn[c * rn : (c + 1) * rn],
            "ni": ni[c * rn : (c + 1) * rn],
        }
        for c in range(N_CORES)
    ]
    res = run_bass_kernel_spmd(_get_nc(), in_maps, list(range(N_CORES)))
    total = np.float64(0.0)
    for c in range(N_CORES):
        total += res.results[c]["out"].sum(dtype=np.float64)
    return np.float32(total)


# revision 4
# speedup vs baseline: 1.2108x; 1.2108x over previous
"""Trainium2 Bass kernel for the contrastive loss:

    epos = exp(cos_sim(q_pos, img_pos))   # [2B] rows, D=1024
    eneg = exp(cos_sim(q_neg, img_neg))   # [23B]
    pos_sum = segsum(epos, 2); neg_sum = segsum(eneg, 23)   # [B]
    loss = sum((neg_sum - pos_sum) / (pos_sum + neg_sum + 0.001))

Data-parallel over 8 NeuronCores: core c takes batch items [c*512, (c+1)*512),
i.e. rows [c*1024,(c+1)*1024) of the pos tensors and [c*11776,(c+1)*11776) of
the neg tensors. Each core emits its 512 per-item values; the host sums.

Per-core layout: local item i = 4*p + s (partition p in [0,128), slot s in
[0,4)), so partition p owns pos rows 8p..8p+7 and neg rows 92p..92p+91 of the
core's shard — each partition's rows are contiguous in DRAM, so every DMA is
128 partitions x (4 rows * 4KiB) contiguous.

Per 128-row slice [128, 1024]: the row-wise dot runs on the vector engine as
one fused scalar_tensor_tensor ((a*1)*b with accum_out), and the two
sum-of-squares run on the scalar engine as Square activations with accum_out.
A fraction of the b-squares is moved to the vector engine to balance the two
engines; both stay below the DMA floor (~100 MiB/core through 16 SDMA
engines).

cos and e=exp(cos) are computed per chunk as stats complete, using
1/sqrt(x) = exp(-0.5*ln(x)) so the scalar engine needs only the
natural_log_exp_and_others table set (square/ln/exp) for the entire kernel —
no ~2.7us ACT table switches in the final tail. The tail is just the two
segmented reductions and the per-item fixup.
"""

import numpy as np

import concourse.bass as bass
import concourse.tile as tile
from concourse import mybir
from concourse.bass_utils import run_bass_kernel_spmd

EPS_COS = 1e-8
EP = 0.001

N_CORES = 8
P = 128            # SBUF partitions
D = 1024           # embedding dim
B_FULL = 4096      # total batch items
ITEMS = B_FULL // N_CORES   # 512 items per core
SLOTS = ITEMS // P          # 4 items per partition
J_POS = SLOTS * 2           # 8 pos rows per partition
J_NEG = SLOTS * 23          # 92 neg rows per partition
G = 4                       # j-slices per DMA chunk (2 MiB per tensor)

F32 = mybir.dt.float32
ALU = mybir.AluOpType
ACTF = mybir.ActivationFunctionType


def _split_multiwait_instructions(nc):
    """The walrus build here rejects >1 sync-wait per instruction; hoist extra
    waits onto single-wait NOPs placed just before the instruction."""
    ctr = 0
    for fn in nc.m.functions:
        for bb in fn.blocks:
            insts = list(bb.instructions)
            if not any(
                i.sync_info is not None and len(i.sync_info.on_wait) > 1
                for i in insts
            ):
                continue
            new_insts = []
            for inst in insts:
                si = inst.sync_info
                if si is not None and len(si.on_wait) > 1:
                    waits = list(si.on_wait)
                    is_drain = type(inst).__name__ == "InstDrain"
                    keep = [] if is_drain else waits[-1:]
                    move = waits if is_drain else waits[:-1]
                    for w in move:
                        ctr += 1
                        new_insts.append(
                            mybir.InstNoOp(
                                name=f"I-wsplit-{ctr}",
                                engine=inst.engine,
                                sync_info=mybir.SyncInfo(on_wait=[w], on_update=[]),
                                text_hint="wsplit",
                            )
                        )
                    si.on_wait = keep
                new_insts.append(inst)
            bb.instructions = new_insts


def build_bass():
    nc = bass.Bass()
    qp = nc.declare_dram_parameter("qp", [P * J_POS, D], F32, isOutput=False)
    pi = nc.declare_dram_parameter("pi", [P * J_POS, D], F32, isOutput=False)
    qn = nc.declare_dram_parameter("qn", [P * J_NEG, D], F32, isOutput=False)
    ni = nc.declare_dram_parameter("ni", [P * J_NEG, D], F32, isOutput=False)
    out = nc.declare_dram_parameter("out", [P, SLOTS], F32, isOutput=True)

    qp_v = qp[:].rearrange("(p j) d -> p j d", j=J_POS)
    pi_v = pi[:].rearrange("(p j) d -> p j d", j=J_POS)
    qn_v = qn[:].rearrange("(p j) d -> p j d", j=J_NEG)
    ni_v = ni[:].rearrange("(p j) d -> p j d", j=J_NEG)

    with tile.TileContext(nc) as tc:
        with (
            tc.tile_pool(name="io", bufs=4) as io,
            tc.tile_pool(name="st", bufs=1) as st,
            tc.tile_pool(name="cols", bufs=3) as cols,
        ):
            dot_p = st.tile([P, J_POS], F32)
            na2_p = st.tile([P, J_POS], F32)
            nb2_p = st.tile([P, J_POS], F32)
            dot_n = st.tile([P, J_NEG], F32)
            na2_n = st.tile([P, J_NEG], F32)
            nb2_n = st.tile([P, J_NEG], F32)
            e_p = st.tile([P, J_POS], F32)
            e_n = st.tile([P, J_NEG], F32)
            scr_v = st.tile([P, D], F32)
            scr_s = st.tile([P, D], F32)

            # (a_view, b_view, dots, na2, nb2, e, j0) per chunk of G slices
            chunks = []
            for c in range(J_POS // G):
                chunks.append((qp_v, pi_v, dot_p, na2_p, nb2_p, e_p, c * G))
            for c in range(J_NEG // G):
                chunks.append((qn_v, ni_v, dot_n, na2_n, nb2_n, e_n, c * G))

            slice_idx = 0
            for a_v, b_v, dots, na2s, nb2s, e_out, j0 in chunks:
                a_t = io.tile([P, G, D], F32, tag="a")
                b_t = io.tile([P, G, D], F32, tag="b")
                nc.sync.dma_start(out=a_t, in_=a_v[:, j0 : j0 + G, :])
                nc.sync.dma_start(out=b_t, in_=b_v[:, j0 : j0 + G, :])
                for g in range(G):
                    j = j0 + g
                    a_sl = a_t[:, g, :]
                    b_sl = b_t[:, g, :]
                    nc.vector.scalar_tensor_tensor(
                        out=scr_v[:], in0=a_sl, scalar=1.0, in1=b_sl,
                        op0=ALU.mult, op1=ALU.mult,
                        accum_out=dots[:, j : j + 1],
                    )
                    nc.scalar.activation(
                        out=scr_s[:], in_=a_sl, func=ACTF.Square,
                        accum_out=na2s[:, j : j + 1],
                    )
                    # ~3/8 of b-squares on the vector engine balances DVE/ACT
                    if (slice_idx % 8) < 3:
                        nc.vector.scalar_tensor_tensor(
                            out=scr_v[:], in0=b_sl, scalar=1.0, in1=b_sl,
                            op0=ALU.mult, op1=ALU.mult,
                            accum_out=nb2s[:, j : j + 1],
                        )
                    else:
                        nc.scalar.activation(
                            out=scr_s[:], in_=b_sl, func=ACTF.Square,
                            accum_out=nb2s[:, j : j + 1],
                        )
                    slice_idx += 1

                # e[:, j0:j0+G] = exp(dot * exp(-0.5*ln(max(na2*nb2, eps^2))))
                dsl = slice(j0, j0 + G)
                prod = cols.tile([P, G], F32, tag="prod")
                nc.vector.tensor_tensor(
                    out=prod[:], in0=na2s[:, dsl], in1=nb2s[:, dsl], op=ALU.mult
                )
                nc.vector.tensor_scalar(
                    out=prod[:], in0=prod[:], scalar1=EPS_COS * EPS_COS,
                    scalar2=None, op0=ALU.max,
                )
                nc.scalar.activation(out=prod[:], in_=prod[:], func=ACTF.Ln)
                nc.scalar.activation(
                    out=prod[:], in_=prod[:], func=ACTF.Exp, scale=-0.5
                )
                cosv = cols.tile([P, G], F32, tag="cos")
                nc.vector.tensor_tensor(
                    out=cosv[:], in0=dots[:, dsl], in1=prod[:], op=ALU.mult
                )
                nc.scalar.activation(
                    out=e_out[:, dsl], in_=cosv[:], func=ACTF.Exp
                )

            pos_sum = st.tile([P, SLOTS], F32)
            neg_sum = st.tile([P, SLOTS], F32)
            nc.vector.tensor_reduce(
                out=pos_sum[:],
                in_=e_p[:].rearrange("p (s t) -> p s t", t=2),
                axis=mybir.AxisListType.X,
                op=ALU.add,
            )
            nc.vector.tensor_reduce(
                out=neg_sum[:],
                in_=e_n[:].rearrange("p (s t) -> p s t", t=23),
                axis=mybir.AxisListType.X,
                op=ALU.add,
            )
            num = st.tile([P, SLOTS], F32)
            den = st.tile([P, SLOTS], F32)
            nc.vector.tensor_tensor(
                out=num[:], in0=neg_sum[:], in1=pos_sum[:], op=ALU.subtract
            )
            nc.vector.tensor_tensor(
                out=den[:], in0=neg_sum[:], in1=pos_sum[:], op=ALU.add
            )
            nc.vector.tensor_scalar(
                out=den[:], in0=den[:], scalar1=EP, scalar2=None, op0=ALU.add
            )
            rden = st.tile([P, SLOTS], F32)
            nc.vector.reciprocal(out=rden[:], in_=den[:])
            per_item = st.tile([P, SLOTS], F32)
            nc.vector.tensor_tensor(
                out=per_item[:], in0=num[:], in1=rden[:], op=ALU.mult
            )
            nc.sync.dma_start(out=out[:], in_=per_item[:])

    _split_multiwait_instructions(nc)
    return nc


_NC_CACHE = None


def _get_nc():
    global _NC_CACHE
    if _NC_CACHE is None:
        _NC_CACHE = build_bass()
    return _NC_CACHE


def kernel(question_embeddings_pos, question_embeddings_neg,
           pos_image_embeddings, neg_image_embeddings, batch_size=None,
           **_unused):
    qp = np.ascontiguousarray(np.asarray(question_embeddings_pos, dtype=np.float32))
    qn = np.ascontiguousarray(np.asarray(question_embeddings_neg, dtype=np.float32))
    pi = np.ascontiguousarray(np.asarray(pos_image_embeddings, dtype=np.float32))
    ni = np.ascontiguousarray(np.asarray(neg_image_embeddings, dtype=np.float32))

    rp = 2 * ITEMS   # pos rows per core
    rn = 23 * ITEMS  # neg rows per core
    in_maps = [
        {
            "qp": qp[c * rp : (c + 1) * rp],
            "pi": pi[c * rp : (c + 1) * rp],
            "qn": qn[c * rn : (c + 1) * rn],
            "ni": ni[c * rn : (c + 1) * rn],
        }
        for c in range(N_CORES)
    ]
    res = run_bass_kernel_spmd(_get_nc(), in_maps, list(range(N_CORES)))
    total = np.float64(0.0)
    for c in range(N_CORES):
        total += res.results[c]["out"].sum(dtype=np.float64)
    return np.float32(total)


# revision 7
# speedup vs baseline: 1.2324x; 1.0178x over previous
"""Trainium2 Bass kernel for the contrastive loss:

    epos = exp(cos_sim(q_pos, img_pos))   # [2B] rows, D=1024
    eneg = exp(cos_sim(q_neg, img_neg))   # [23B]
    pos_sum = segsum(epos, 2); neg_sum = segsum(eneg, 23)   # [B]
    loss = sum((neg_sum - pos_sum) / (pos_sum + neg_sum + 0.001))

Data-parallel over 8 NeuronCores: core c takes batch items [c*512, (c+1)*512),
i.e. rows [c*1024,(c+1)*1024) of the pos tensors and [c*11776,(c+1)*11776) of
the neg tensors. Each core emits its 512 per-item values; the host sums.

Per-core layout: local item i = 4*p + s (partition p in [0,128), slot s in
[0,4)), so partition p owns pos rows 8p..8p+7 and neg rows 92p..92p+91 of the
core's shard — each partition's rows are contiguous in DRAM, so every DMA is
128 partitions x (4 rows * 4KiB) contiguous.

Per 128-row slice [128, 1024]: the row-wise dot runs on the vector engine as
one fused scalar_tensor_tensor ((a*1)*b with accum_out), and the two
sum-of-squares run on the scalar engine as Square activations with accum_out.
A fraction of the b-squares is moved to the vector engine to balance the two
engines; both stay below the DMA floor (~100 MiB/core through 16 SDMA
engines).

cos and e=exp(cos) are computed per chunk as stats complete, using
1/sqrt(x) = exp(-0.5*ln(x)) so the scalar engine needs only the
natural_log_exp_and_others table set (square/ln/exp) for the entire kernel —
no ~2.7us ACT table switches in the final tail. The tail is just the two
segmented reductions and the per-item fixup.
"""

import numpy as np

import concourse.bass as bass
import concourse.tile as tile
from concourse import mybir
from concourse.bass_utils import run_bass_kernel_spmd

EPS_COS = 1e-8
EP = 0.001

N_CORES = 8
P = 128            # SBUF partitions
D = 1024           # embedding dim
B_FULL = 4096      # total batch items
ITEMS = B_FULL // N_CORES   # 512 items per core
SLOTS = ITEMS // P          # 4 items per partition
J_POS = SLOTS * 2           # 8 pos rows per partition
J_NEG = SLOTS * 23          # 92 neg rows per partition
G = 4                       # j-slices per DMA chunk (2 MiB per tensor)

F32 = mybir.dt.float32
ALU = mybir.AluOpType
ACTF = mybir.ActivationFunctionType


def _split_multiwait_instructions(nc):
    """The walrus build here rejects >1 sync-wait per instruction; hoist extra
    waits onto single-wait NOPs placed just before the instruction."""
    ctr = 0
    for fn in nc.m.functions:
        for bb in fn.blocks:
            insts = list(bb.instructions)
            if not any(
                i.sync_info is not None and len(i.sync_info.on_wait) > 1
                for i in insts
            ):
                continue
            new_insts = []
            for inst in insts:
                si = inst.sync_info
                if si is not None and len(si.on_wait) > 1:
                    waits = list(si.on_wait)
                    is_drain = type(inst).__name__ == "InstDrain"
                    keep = [] if is_drain else waits[-1:]
                    move = waits if is_drain else waits[:-1]
                    for w in move:
                        ctr += 1
                        new_insts.append(
                            mybir.InstNoOp(
                                name=f"I-wsplit-{ctr}",
                                engine=inst.engine,
                                sync_info=mybir.SyncInfo(on_wait=[w], on_update=[]),
                                text_hint="wsplit",
                            )
                        )
                    si.on_wait = keep
                new_insts.append(inst)
            bb.instructions = new_insts


def build_bass():
    nc = bass.Bass()
    qp = nc.declare_dram_parameter("qp", [P * J_POS, D], F32, isOutput=False)
    pi = nc.declare_dram_parameter("pi", [P * J_POS, D], F32, isOutput=False)
    qn = nc.declare_dram_parameter("qn", [P * J_NEG, D], F32, isOutput=False)
    ni = nc.declare_dram_parameter("ni", [P * J_NEG, D], F32, isOutput=False)
    out = nc.declare_dram_parameter("out", [P, SLOTS], F32, isOutput=True)

    qp_v = qp[:].rearrange("(p j) d -> p j d", j=J_POS)
    pi_v = pi[:].rearrange("(p j) d -> p j d", j=J_POS)
    qn_v = qn[:].rearrange("(p j) d -> p j d", j=J_NEG)
    ni_v = ni[:].rearrange("(p j) d -> p j d", j=J_NEG)

    with tile.TileContext(nc) as tc:
        with (
            tc.tile_pool(name="io", bufs=4) as io,
            tc.tile_pool(name="st", bufs=1) as st,
            tc.tile_pool(name="cols", bufs=3) as cols,
        ):
            dot_p = st.tile([P, J_POS], F32)
            na2_p = st.tile([P, J_POS], F32)
            nb2_p = st.tile([P, J_POS], F32)
            dot_n = st.tile([P, J_NEG], F32)
            na2_n = st.tile([P, J_NEG], F32)
            nb2_n = st.tile([P, J_NEG], F32)
            e_p = st.tile([P, J_POS], F32)
            e_n = st.tile([P, J_NEG], F32)
            scr_v = st.tile([P, D], F32)
            scr_s = st.tile([P, D], F32)

            # (a_view, b_view, dots, na2, nb2, e, j0, g) per chunk; the last
            # chunks shrink (4,...,4,2,1,1) so the serial compute after the
            # final input load is as short as possible.
            def chunk_sizes(total):
                sizes = []
                rem = total
                for tail_g in (1, 1, 2):
                    if rem > tail_g:
                        rem -= tail_g
                assert rem % G == 0
                sizes = [G] * (rem // G)
                if total - rem == 4:
                    sizes += [2, 1, 1]
                else:
                    assert total == rem
                return sizes

            chunks = []
            for view_a, view_b, d_t, a_t_, b_t_, e_t, total in (
                (qp_v, pi_v, dot_p, na2_p, nb2_p, e_p, J_POS),
                (qn_v, ni_v, dot_n, na2_n, nb2_n, e_n, J_NEG),
            ):
                j0 = 0
                for gsz in chunk_sizes(total):
                    chunks.append((view_a, view_b, d_t, a_t_, b_t_, e_t, j0, gsz))
                    j0 += gsz
                assert j0 == total

            n_chunks = len(chunks)
            slice_idx = 0
            for ci, (a_v, b_v, dots, na2s, nb2s, e_out, j0, gsz) in enumerate(chunks):
                a_t = io.tile([P, G, D], F32, tag="a")
                b_t = io.tile([P, G, D], F32, tag="b")
                nc.sync.dma_start(out=a_t[:, :gsz, :], in_=a_v[:, j0 : j0 + gsz, :])
                nc.sync.dma_start(out=b_t[:, :gsz, :], in_=b_v[:, j0 : j0 + gsz, :])
                last_chunks = ci >= n_chunks - 2
                for g in range(gsz):
                    j = j0 + g
                    a_sl = a_t[:, g, :]
                    b_sl = b_t[:, g, :]
                    nc.vector.scalar_tensor_tensor(
                        out=scr_v[:], in0=a_sl, scalar=1.0, in1=b_sl,
                        op0=ALU.mult, op1=ALU.mult,
                        accum_out=dots[:, j : j + 1],
                    )
                    # ~3/8 of b-squares on the vector engine balances DVE/ACT;
                    # in the final chunks split the three ops across engines to
                    # minimize the post-stream serial chain.
                    sq_b_on_dve = (
                        True if last_chunks else (slice_idx % 8) < 3
                    )
                    nc.scalar.activation(
                        out=scr_s[:], in_=a_sl, func=ACTF.Square,
                        accum_out=na2s[:, j : j + 1],
                    )
                    if sq_b_on_dve:
                        nc.vector.scalar_tensor_tensor(
                            out=scr_v[:], in0=b_sl, scalar=1.0, in1=b_sl,
                            op0=ALU.mult, op1=ALU.mult,
                            accum_out=nb2s[:, j : j + 1],
                        )
                    else:
                        nc.scalar.activation(
                            out=scr_s[:], in_=b_sl, func=ACTF.Square,
                            accum_out=nb2s[:, j : j + 1],
                        )
                    slice_idx += 1

                # e[:, j0:j0+gsz] = exp(dot * exp(-0.5*ln(max(na2*nb2, eps^2))))
                dsl = slice(j0, j0 + gsz)
                prod_t = cols.tile([P, G], F32, tag="prod")
                prod = prod_t[:, :gsz]
                nc.vector.tensor_tensor(
                    out=prod[:], in0=na2s[:, dsl], in1=nb2s[:, dsl], op=ALU.mult
                )
                nc.vector.tensor_scalar(
                    out=prod[:], in0=prod[:], scalar1=EPS_COS * EPS_COS,
                    scalar2=None, op0=ALU.max,
                )
                nc.scalar.activation(out=prod[:], in_=prod[:], func=ACTF.Ln)
                nc.scalar.activation(
                    out=prod[:], in_=prod[:], func=ACTF.Exp, scale=-0.5
                )
                cosv_t = cols.tile([P, G], F32, tag="cos")
                cosv = cosv_t[:, :gsz]
                nc.vector.tensor_tensor(
                    out=cosv[:], in0=dots[:, dsl], in1=prod[:], op=ALU.mult
                )
                nc.scalar.activation(
                    out=e_out[:, dsl], in_=cosv[:], func=ACTF.Exp
                )

            pos_sum = st.tile([P, SLOTS], F32)
            neg_sum = st.tile([P, SLOTS], F32)
            nc.vector.tensor_reduce(
                out=pos_sum[:],
                in_=e_p[:].rearrange("p (s t) -> p s t", t=2),
                axis=mybir.AxisListType.X,
                op=ALU.add,
            )
            nc.vector.tensor_reduce(
                out=neg_sum[:],
                in_=e_n[:].rearrange("p (s t) -> p s t", t=23),
                axis=mybir.AxisListType.X,
                op=ALU.add,
            )
            num = st.tile([P, SLOTS], F32)
            den = st.tile([P, SLOTS], F32)
            nc.vector.tensor_tensor(
                out=num[:], in0=neg_sum[:], in1=pos_sum[:], op=ALU.subtract
            )
            nc.vector.tensor_tensor(
                out=den[:], in0=neg_sum[:], in1=pos_sum[:], op=ALU.add
            )
            nc.vector.tensor_scalar(
                out=den[:], in0=den[:], scalar1=EP, scalar2=None, op0=ALU.add
            )
            rden = st.tile([P, SLOTS], F32)
            nc.vector.reciprocal(out=rden[:], in_=den[:])
            per_item = st.tile([P, SLOTS], F32)
            nc.vector.tensor_tensor(
                out=per_item[:], in0=num[:], in1=rden[:], op=ALU.mult
            )
            nc.sync.dma_start(out=out[:], in_=per_item[:])

    _split_multiwait_instructions(nc)
    return nc


_NC_CACHE = None


def _get_nc():
    global _NC_CACHE
    if _NC_CACHE is None:
        _NC_CACHE = build_bass()
    return _NC_CACHE


def kernel(question_embeddings_pos, question_embeddings_neg,
           pos_image_embeddings, neg_image_embeddings, batch_size=None,
           **_unused):
    qp = np.ascontiguousarray(np.asarray(question_embeddings_pos, dtype=np.float32))
    qn = np.ascontiguousarray(np.asarray(question_embeddings_neg, dtype=np.float32))
    pi = np.ascontiguousarray(np.asarray(pos_image_embeddings, dtype=np.float32))
    ni = np.ascontiguousarray(np.asarray(neg_image_embeddings, dtype=np.float32))

    rp = 2 * ITEMS   # pos rows per core
    rn = 23 * ITEMS  # neg rows per core
    in_maps = [
        {
            "qp": qp[c * rp : (c + 1) * rp],
            "pi": pi[c * rp : (c + 1) * rp],
            "qn": qn[c * rn : (c + 1) * rn],
            "ni": ni[c * rn : (c + 1) * rn],
        }
        for c in range(N_CORES)
    ]
    res = run_bass_kernel_spmd(_get_nc(), in_maps, list(range(N_CORES)))
    total = np.float64(0.0)
    for c in range(N_CORES):
        total += res.results[c]["out"].sum(dtype=np.float64)
    return np.float32(total)


# revision 8
# speedup vs baseline: 1.3788x; 1.1188x over previous
"""Trainium2 Bass kernel for the contrastive loss:

    epos = exp(cos_sim(q_pos, img_pos))   # [2B] rows, D=1024
    eneg = exp(cos_sim(q_neg, img_neg))   # [23B]
    pos_sum = segsum(epos, 2); neg_sum = segsum(eneg, 23)   # [B]
    loss = sum((neg_sum - pos_sum) / (pos_sum + neg_sum + 0.001))

Data-parallel over 8 NeuronCores: core c takes batch items [c*512, (c+1)*512),
i.e. rows [c*1024,(c+1)*1024) of the pos tensors and [c*11776,(c+1)*11776) of
the neg tensors. Each core emits its 512 per-item values; the host sums.

Per-core layout: local item i = 4*p + s (partition p in [0,128), slot s in
[0,4)), so partition p owns pos rows 8p..8p+7 and neg rows 92p..92p+91 of the
core's shard — each partition's rows are contiguous in DRAM, so every DMA is
128 partitions x (4 rows * 4KiB) contiguous.

Per 128-row slice [128, 1024]: the row-wise dot runs on the vector engine as
one fused scalar_tensor_tensor ((a*1)*b with accum_out), and the two
sum-of-squares run on the scalar engine as Square activations with accum_out.
A fraction of the b-squares is moved to the vector engine to balance the two
engines; both stay below the DMA floor (~100 MiB/core through 16 SDMA
engines).

cos and e=exp(cos) are computed per chunk as stats complete, using
1/sqrt(x) = exp(-0.5*ln(x)) so the scalar engine needs only the
natural_log_exp_and_others table set (square/ln/exp) for the entire kernel —
no ~2.7us ACT table switches in the final tail. The tail is just the two
segmented reductions and the per-item fixup.
"""

import numpy as np

import concourse.bass as bass
import concourse.tile as tile
from concourse import mybir
from concourse.bass_utils import run_bass_kernel_spmd

EPS_COS = 1e-8
EP = 0.001

N_CORES = 8
P = 128            # SBUF partitions
D = 1024           # embedding dim
B_FULL = 4096      # total batch items
ITEMS = B_FULL // N_CORES   # 512 items per core
SLOTS = ITEMS // P          # 4 items per partition
J_POS = SLOTS * 2           # 8 pos rows per partition
J_NEG = SLOTS * 23          # 92 neg rows per partition
G = 4                       # j-slices per DMA chunk (2 MiB per tensor)

F32 = mybir.dt.float32
ALU = mybir.AluOpType
ACTF = mybir.ActivationFunctionType


def _split_multiwait_instructions(nc):
    """The walrus build here rejects >1 sync-wait per instruction; hoist extra
    waits onto single-wait NOPs placed just before the instruction."""
    ctr = 0
    for fn in nc.m.functions:
        for bb in fn.blocks:
            insts = list(bb.instructions)
            if not any(
                i.sync_info is not None and len(i.sync_info.on_wait) > 1
                for i in insts
            ):
                continue
            new_insts = []
            for inst in insts:
                si = inst.sync_info
                if si is not None and len(si.on_wait) > 1:
                    waits = list(si.on_wait)
                    is_drain = type(inst).__name__ == "InstDrain"
                    keep = [] if is_drain else waits[-1:]
                    move = waits if is_drain else waits[:-1]
                    for w in move:
                        ctr += 1
                        new_insts.append(
                            mybir.InstNoOp(
                                name=f"I-wsplit-{ctr}",
                                engine=inst.engine,
                                sync_info=mybir.SyncInfo(on_wait=[w], on_update=[]),
                                text_hint="wsplit",
                            )
                        )
                    si.on_wait = keep
                new_insts.append(inst)
            bb.instructions = new_insts


def build_bass():
    nc = bass.Bass()
    qp = nc.declare_dram_parameter("qp", [P * J_POS, D], F32, isOutput=False)
    pi = nc.declare_dram_parameter("pi", [P * J_POS, D], F32, isOutput=False)
    qn = nc.declare_dram_parameter("qn", [P * J_NEG, D], F32, isOutput=False)
    ni = nc.declare_dram_parameter("ni", [P * J_NEG, D], F32, isOutput=False)
    out = nc.declare_dram_parameter("out", [P, SLOTS], F32, isOutput=True)

    qp_v = qp[:].rearrange("(p j) d -> p j d", j=J_POS)
    pi_v = pi[:].rearrange("(p j) d -> p j d", j=J_POS)
    qn_v = qn[:].rearrange("(p j) d -> p j d", j=J_NEG)
    ni_v = ni[:].rearrange("(p j) d -> p j d", j=J_NEG)

    with tile.TileContext(nc) as tc:
        with (
            tc.tile_pool(name="io", bufs=4) as io,
            tc.tile_pool(name="st", bufs=1) as st,
        ):
            J_ALL = J_POS + J_NEG   # pos stats in cols [0,8), neg in [8,100)
            dot_all = st.tile([P, J_ALL], F32)
            na2_all = st.tile([P, J_ALL], F32)
            nb2_all = st.tile([P, J_ALL], F32)
            e_all = st.tile([P, J_ALL], F32)
            scr_v = st.tile([P, D], F32)
            scr_s = st.tile([P, D], F32)

            # Chunk schedule: the last chunks shrink (...,4,2,1,1) so the
            # serial compute after the final input load is minimal.
            def chunk_sizes(total, shrink_tail):
                if not shrink_tail:
                    assert total % G == 0
                    return [G] * (total // G)
                rem = total - 4
                assert rem % G == 0
                return [G] * (rem // G) + [2, 1, 1]

            chunks = []   # (a_view, b_view, col0, j0, gsz)
            for view_a, view_b, col0, total, shrink in (
                (qp_v, pi_v, 0, J_POS, False),
                (qn_v, ni_v, J_POS, J_NEG, True),
            ):
                j0 = 0
                for gsz in chunk_sizes(total, shrink):
                    chunks.append((view_a, view_b, col0, j0, gsz))
                    j0 += gsz
                assert j0 == total

            # Streaming phase: only dots + squares, no cross-engine chains.
            # 3/11 of b-squares go to the vector engine: unit cost is
            # ~1.22us/slice on DVE vs ~1.30us on ACT (ACT pays a 185ns
            # ACTIVATION_READ_ACCUMULATOR per accumulate).
            slice_idx = 0
            for a_v, b_v, col0, j0, gsz in chunks:
                a_t = io.tile([P, G, D], F32, tag="a")
                b_t = io.tile([P, G, D], F32, tag="b")
                nc.sync.dma_start(out=a_t[:, :gsz, :], in_=a_v[:, j0 : j0 + gsz, :])
                nc.sync.dma_start(out=b_t[:, :gsz, :], in_=b_v[:, j0 : j0 + gsz, :])
                for g in range(gsz):
                    j = col0 + j0 + g
                    a_sl = a_t[:, g, :]
                    b_sl = b_t[:, g, :]
                    nc.vector.scalar_tensor_tensor(
                        out=scr_v[:], in0=a_sl, scalar=1.0, in1=b_sl,
                        op0=ALU.mult, op1=ALU.mult,
                        accum_out=dot_all[:, j : j + 1],
                    )
                    nc.scalar.activation(
                        out=scr_s[:], in_=a_sl, func=ACTF.Square,
                        accum_out=na2_all[:, j : j + 1],
                    )
                    if (slice_idx % 11) < 3:
                        nc.vector.scalar_tensor_tensor(
                            out=scr_v[:], in0=b_sl, scalar=1.0, in1=b_sl,
                            op0=ALU.mult, op1=ALU.mult,
                            accum_out=nb2_all[:, j : j + 1],
                        )
                    else:
                        nc.scalar.activation(
                            out=scr_s[:], in_=b_sl, func=ACTF.Square,
                            accum_out=nb2_all[:, j : j + 1],
                        )
                    slice_idx += 1

            # Batched tail: e = exp(dot * exp(-0.5*ln(max(na2*nb2, eps^2))))
            # over all 100 columns at once; ln/exp share the square table set
            # so no ACT table switches happen here.
            prod = st.tile([P, J_ALL], F32)
            nc.vector.tensor_tensor(
                out=prod[:], in0=na2_all[:], in1=nb2_all[:], op=ALU.mult
            )
            nc.vector.tensor_scalar(
                out=prod[:], in0=prod[:], scalar1=EPS_COS * EPS_COS,
                scalar2=None, op0=ALU.max,
            )
            nc.scalar.activation(out=prod[:], in_=prod[:], func=ACTF.Ln)
            nc.scalar.activation(
                out=prod[:], in_=prod[:], func=ACTF.Exp, scale=-0.5
            )
            cosv = st.tile([P, J_ALL], F32)
            nc.vector.tensor_tensor(
                out=cosv[:], in0=dot_all[:], in1=prod[:], op=ALU.mult
            )
            nc.scalar.activation(out=e_all[:], in_=cosv[:], func=ACTF.Exp)

            pos_sum = st.tile([P, SLOTS], F32)
            neg_sum = st.tile([P, SLOTS], F32)
            nc.vector.tensor_reduce(
                out=pos_sum[:],
                in_=e_all[:, :J_POS].rearrange("p (s t) -> p s t", t=2),
                axis=mybir.AxisListType.X,
                op=ALU.add,
            )
            nc.vector.tensor_reduce(
                out=neg_sum[:],
                in_=e_all[:, J_POS:].rearrange("p (s t) -> p s t", t=23),
                axis=mybir.AxisListType.X,
                op=ALU.add,
            )
            num = st.tile([P, SLOTS], F32)
            den = st.tile([P, SLOTS], F32)
            nc.vector.tensor_tensor(
                out=num[:], in0=neg_sum[:], in1=pos_sum[:], op=ALU.subtract
            )
            nc.vector.tensor_tensor(
                out=den[:], in0=neg_sum[:], in1=pos_sum[:], op=ALU.add
            )
            nc.vector.tensor_scalar(
                out=den[:], in0=den[:], scalar1=EP, scalar2=None, op0=ALU.add
            )
            rden = st.tile([P, SLOTS], F32)
            nc.vector.reciprocal(out=rden[:], in_=den[:])
            per_item = st.tile([P, SLOTS], F32)
            nc.vector.tensor_tensor(
                out=per_item[:], in0=num[:], in1=rden[:], op=ALU.mult
            )
            nc.sync.dma_start(out=out[:], in_=per_item[:])

    _split_multiwait_instructions(nc)
    return nc


_NC_CACHE = None


def _get_nc():
    global _NC_CACHE
    if _NC_CACHE is None:
        _NC_CACHE = build_bass()
    return _NC_CACHE


def kernel(question_embeddings_pos, question_embeddings_neg,
           pos_image_embeddings, neg_image_embeddings, batch_size=None,
           **_unused):
    qp = np.ascontiguousarray(np.asarray(question_embeddings_pos, dtype=np.float32))
    qn = np.ascontiguousarray(np.asarray(question_embeddings_neg, dtype=np.float32))
    pi = np.ascontiguousarray(np.asarray(pos_image_embeddings, dtype=np.float32))
    ni = np.ascontiguousarray(np.asarray(neg_image_embeddings, dtype=np.float32))

    rp = 2 * ITEMS   # pos rows per core
    rn = 23 * ITEMS  # neg rows per core
    in_maps = [
        {
            "qp": qp[c * rp : (c + 1) * rp],
            "pi": pi[c * rp : (c + 1) * rp],
            "qn": qn[c * rn : (c + 1) * rn],
            "ni": ni[c * rn : (c + 1) * rn],
        }
        for c in range(N_CORES)
    ]
    res = run_bass_kernel_spmd(_get_nc(), in_maps, list(range(N_CORES)))
    total = np.float64(0.0)
    for c in range(N_CORES):
        total += res.results[c]["out"].sum(dtype=np.float64)
    return np.float32(total)
